# revision 2
# baseline (speedup 1.0000x reference)
"""GQA multi-head attention (b=2, s=2048, d=2048, 32 Q heads / 8 KV heads,
head_dim=64, RoPE, causal) on 8 Trainium2 NeuronCores.

Sharding: tensor-parallel over heads x data-parallel over batch.
Core c = 4*bi + g handles batch bi and head-group g (8 Q heads, 2 KV heads).
Each core computes a partial [2048, 2048] output (its head block times the
matching wo rows); the host sums the 4 partials per batch.

v2 design notes (vs the original 3-phase kernel):
  - All matmul operands are bf16 (PSUM accumulation stays f32): same PE
    throughput as f32r at N>=256 but half the DMA/SBUF traffic, and DVE
    element-wise ops on packed bf16 SBUF operands run at 4x.
  - Phase A does all projections chunk-by-chunk; PSUM evacuation goes to the
    (otherwise idle) Pool engine as f32->bf16 copies, RoPE runs on DVE fully
    in bf16 SBUF.
  - Phase B flattens attention into a global stream of "score units"
    (2 sk-tiles: 2 matmuls + exp + causal masks) and "PV units", software-
    pipelined with lag 1 so the PE never waits on the Activation engine's
    exp. wo output-projection units are injected at block boundaries as PE
    filler (chunk c's wo runs inside chunk c+1's attention stream), keeping
    the PE busy while Act catches up on exp.
  - Softmax denominators come from an extra ones-column in the PV stationary
    (row 64 of the PV output); no max-subtraction (scores are O(10)).
"""

import sys

if "/opt/trn_rl_repo" not in sys.path:
    sys.path.insert(0, "/opt/trn_rl_repo")

import numpy as np
import ml_dtypes

import concourse.bass as bass  # noqa: F401  (import keeps bass registered)
import concourse.tile as tile
from concourse import bacc, mybir
from concourse.bass_utils import run_bass_kernel_spmd

BF16 = mybir.dt.bfloat16
F32 = mybir.dt.float32
NPBF16 = ml_dtypes.bfloat16

S = 2048
D = 2048
NH = 32
NKV = 8
DH = 64
ROPE_BASE = 10000.0
N_CORES = 8
QH_PER_CORE = 8   # local q heads
KVH_PER_CORE = 2  # local kv heads
DQ = QH_PER_CORE * DH   # 512, per-core q width
DKV = KVH_PER_CORE * DH  # 128, per-core kv width

# module-level knobs the test harness can poke
RUN_KWARGS: dict = {}
LAST_RESULTS = None

_COMPILED = None


def _build(loop_n=1, phases=3, abl=0):
    nc = bacc.Bacc("TRN2", target_bir_lowering=False, debug=False)

    xt_d = nc.dram_tensor("xt", [128, 4 * 8192], BF16, kind="ExternalInput").ap()
    wall_d = nc.dram_tensor("wall", [128, 16 * 768], BF16, kind="ExternalInput").ap()
    wo_d = nc.dram_tensor("wo", [128, 4 * 2048], BF16, kind="ExternalInput").ap()
    cos_d = nc.dram_tensor("cos", [128, S], BF16, kind="ExternalInput").ap()
    sin_d = nc.dram_tensor("sin", [128, S], BF16, kind="ExternalInput").ap()
    tri_d = nc.dram_tensor("tri", [128, 128], BF16, kind="ExternalInput").ap()
    eye_d = nc.dram_tensor("eye", [128, 64], BF16, kind="ExternalInput").ap()
    out_d = nc.dram_tensor("out", [S, D], F32, kind="ExternalOutput").ap()

    import contextlib

    with tile.TileContext(nc) as tc:
        with (
            tc.For_i(0, loop_n, 1) if loop_n > 1 else contextlib.nullcontext()
        ):
            _phases(nc, tc, xt_d, wall_d, wo_d, cos_d, sin_d, tri_d, eye_d, out_d)

    nc.compile()
    return nc


def _phases(nc, tc, xt_d, wall_d, wo_d, cos_d, sin_d, tri_d, eye_d, out_d):
    with (
        tc.tile_pool(name="big", bufs=1) as big,
        tc.tile_pool(name="xtp", bufs=2) as xtp,
        tc.tile_pool(name="evac", bufs=2) as evacp,
        tc.tile_pool(name="rope", bufs=2) as ropep,
        tc.tile_pool(name="prp", bufs=4) as prp,
        tc.tile_pool(name="recp", bufs=2) as recp,
        tc.tile_pool(name="otp", bufs=2) as otp,
    ):
        # ---- persistent SBUF tiles ----
        # DMA order matters for PE start latency: first w k-chunk, then the
        # first xt half (enough for proj k=0..7), then the rest. wo_sb loads
        # at the end of phase A (first consumer is ~150us in).
        w_sb = big.tile([128, 16 * 768], BF16)
        xt0 = xtp.tile([128, 8192], BF16, tag="xt", name="xt0")
        nc.sync.dma_start(w_sb[:, 0:3072], wall_d[:, 0:3072])
        nc.sync.dma_start(xt0[:, 0:2048], xt_d[:, 0:2048])
        nc.sync.dma_start(xt0[:, 2048:4096], xt_d[:, 2048:4096])
        nc.sync.dma_start(w_sb[:, 3072:6144], wall_d[:, 3072:6144])
        nc.sync.dma_start(xt0[:, 4096:6144], xt_d[:, 4096:6144])
        nc.sync.dma_start(xt0[:, 6144:8192], xt_d[:, 6144:8192])
        for kq in range(2, 4):
            nc.sync.dma_start(
                w_sb[:, kq * 3072 : (kq + 1) * 3072],
                wall_d[:, kq * 3072 : (kq + 1) * 3072],
            )
        cos_sb = big.tile([128, S], BF16)
        nc.sync.dma_start(cos_sb[:], cos_d[:])
        sin_sb = big.tile([128, S], BF16)
        nc.sync.dma_start(sin_sb[:], sin_d[:])
        tri_sb = big.tile([128, 128], BF16)
        nc.sync.dma_start(tri_sb[:], tri_d[:])
        eye_sb = big.tile([128, 64], BF16)
        nc.sync.dma_start(eye_sb[:], eye_d[:])
        wo_sb = big.tile([128, 4 * 2048], BF16)

        qrot = big.tile([128, 4 * S], BF16)   # m-tile m at [:, m*S : (m+1)*S]
        krot = big.tile([128, S], BF16)       # kv heads on partitions 0-63/64-127
        vaug = big.tile([128, 2 * 16 * 65], BF16)  # per (kv, sk-tile): [sk, 64+1]
        attn = big.tile([128, 4 * S], BF16)   # normalized attention, qrot layout
        nc.vector.memset(vaug[:], 1.0)        # pre-fill ones columns

        # =================== phase A: projections + rope + v ===================
        with (
            tc.tile_pool(name="psP", bufs=1, space="PSUM") as psP,
            tc.tile_pool(name="psT", bufs=2, space="PSUM") as psT,
        ):
            def rope_emit(dst, src, sc):
                # dst = src*cos + shift32(src)*sin; all bf16 in SBUF (4x DVE)
                cs = cos_sb[:, sc * 512 : (sc + 1) * 512]
                sn = sin_sb[:, sc * 512 : (sc + 1) * 512]
                m1 = ropep.tile([128, 512], BF16, tag="m1", name=f"m1_{sc}")
                m2 = ropep.tile([128, 512], BF16, tag="m2", name=f"m2_{sc}")
                nc.vector.tensor_tensor(m1[:], src[:], cs, op=mybir.AluOpType.mult)
                for q in range(4):
                    a, b = q * 32, (q ^ 1) * 32
                    nc.vector.tensor_tensor(
                        m2[b : b + 32, :],
                        src[a : a + 32, :],
                        sn[a : a + 32, :],
                        op=mybir.AluOpType.mult,
                    )
                nc.vector.tensor_tensor(dst, m1[:], m2[:], op=mybir.AluOpType.add)

            vts = {}

            def emit_transpose_one(sc, j):
                # transpose j of chunk sc: kv = j // 4, r = j % 4
                vt_t = vts[sc]
                kv, r = j // 4, j % 4
                i = 4 * sc + r
                tp = psT.tile([128, 64], BF16, tag="vtp", name=f"vtp{sc}_{kv}{r}")
                nc.tensor.matmul(
                    tp[:],
                    vt_t[kv * 64 : (kv + 1) * 64, r * 128 : (r + 1) * 128],
                    eye_sb[kv * 64 : (kv + 1) * 64, :],
                    is_transpose=True,
                    skip_group_check=True,
                )
                base = (kv * 16 + i) * 65
                nc.vector.tensor_copy(vaug[:, base : base + 64], tp[:])

            for sc in range(4):
                if sc == 0:
                    xt_t = xt0
                else:
                    xt_t = xtp.tile([128, 8192], BF16, tag="xt", name=f"xt{sc}")
                    nc.sync.dma_start(xt_t[:], xt_d[:, sc * 8192 : (sc + 1) * 8192])
                # m-major: each projection column finishes early so its Act
                # evac + DVE rope overlap the remaining columns' matmuls.
                # Chunk 0 runs in four k-quarter passes so the PE starts after
                # only the first w chunk + first xt quarter have landed.
                pss0 = {}
                if sc == 0:
                    for quarter in range(3):
                        for m in range(6):
                            if quarter == 0:
                                pss0[m] = psP.tile([128, 512], F32, tag=f"pj{m}",
                                                   name=f"pj{m}_{sc}")
                            ps = pss0[m]
                            for k in range(quarter * 4, quarter * 4 + 4):
                                nc.tensor.matmul(
                                    ps[:],
                                    lhsT=w_sb[:, k * 768 + m * 128 : k * 768 + (m + 1) * 128],
                                    rhs=xt_t[:, k * 512 : (k + 1) * 512],
                                    start=(k == 0),
                                    stop=False,
                                    skip_group_check=True,
                                )
                for m in range(6):
                    if sc == 0:
                        ps = pss0[m]
                        k_lo = 12
                    else:
                        ps = psP.tile([128, 512], F32, tag=f"pj{m}",
                                      name=f"pj{m}_{sc}")
                        k_lo = 0
                    for k in range(k_lo, 16):
                        nc.tensor.matmul(
                            ps[:],
                            lhsT=w_sb[:, k * 768 + m * 128 : k * 768 + (m + 1) * 128],
                            rhs=xt_t[:, k * 512 : (k + 1) * 512],
                            start=(k == 0),
                            stop=(k == 15),
                            skip_group_check=True,
                        )
                    # interleave the previous chunk's 8 v-transposes
                    if sc > 0 and 1 <= m <= 4:
                        emit_transpose_one(sc - 1, 2 * (m - 1))
                        emit_transpose_one(sc - 1, 2 * (m - 1) + 1)
                    if m < 4:
                        qc = evacp.tile([128, 512], BF16, tag=f"qc{m}",
                                        name=f"qc{m}_{sc}")
                        nc.scalar.copy(qc[:], ps[:])
                        rope_emit(
                            qrot[:, m * S + sc * 512 : m * S + (sc + 1) * 512],
                            qc, sc)
                    elif m == 4:
                        kc = evacp.tile([128, 512], BF16, tag="kc", name=f"kc_{sc}")
                        nc.scalar.copy(kc[:], ps[:])
                        rope_emit(krot[:, sc * 512 : (sc + 1) * 512], kc, sc)
                    else:
                        vt_t = evacp.tile([128, 512], BF16, tag="vt", name=f"vt_{sc}")
                        nc.scalar.copy(vt_t[:], ps[:])
                        vts[sc] = vt_t
            for j in range(8):
                emit_transpose_one(3, j)
            nc.sync.dma_start(wo_sb[:], wo_d[:])

        # =================== phase B: attention + wo stream ===================
        with (
            tc.tile_pool(name="scp", bufs=2, space="PSUM") as scp,
            tc.tile_pool(name="outp", bufs=2, space="PSUM") as outp,
            tc.tile_pool(name="ps3p", bufs=2, space="PSUM") as ps3p,
        ):
            blocks = [(m, sub) for m in range(4) for sub in range(2)]

            steps = []
            for c in range(4):
                n_i = 4 * (c + 1)
                for bidx, (m, sub) in enumerate(blocks):
                    gs = list(range(0, n_i, 2))
                    for gi, g in enumerate(gs):
                        steps.append(
                            dict(c=c, m=m, sub=sub, g=g, n_i=n_i, bidx=bidx,
                                 first=(gi == 0), last=(gi == len(gs) - 1),
                                 blk_last=(bidx == len(blocks) - 1))
                        )

            def emit_S(st):
                c, m, sub, g = st["c"], st["m"], st["sub"], st["g"]
                hb = sub * 64
                sc_t = scp.tile([128, 1024], F32, tag="sc",
                                name=f"sc{c}_{m}{sub}_{g}")
                q_chunk = qrot[hb : hb + 64, m * S + c * 512 : m * S + (c + 1) * 512]
                for j in range(2):
                    i = g + j
                    # causal trim: for diagonal sk-tiles only sq >= 128*r is
                    # unmasked; the skipped score columns are never read (exp
                    # output there is stale-PSUM garbage, but PV skips it too)
                    off = max(0, 128 * (i - 4 * c))
                    nc.tensor.matmul(
                        sc_t[:, j * 512 + off : (j + 1) * 512],
                        lhsT=krot[hb : hb + 64, i * 128 : (i + 1) * 128],
                        rhs=q_chunk[:, off:512],
                        start=True,
                        stop=True,
                        skip_group_check=True,
                    )
                pr_t = prp.tile([128, 1024], BF16, tag="pr",
                                name=f"pr{c}_{m}{sub}_{g}")
                nc.scalar.activation(
                    pr_t[:], sc_t[:], mybir.ActivationFunctionType.Exp, scale=0.125
                )
                for j in range(2):
                    r = (g + j) - 4 * c
                    if r >= 0:
                        lo = j * 512 + 128 * r
                        nc.vector.tensor_tensor(
                            pr_t[:, lo : lo + 128],
                            pr_t[:, lo : lo + 128],
                            tri_sb[:],
                            op=mybir.AluOpType.mult,
                        )
                st["pr"] = pr_t

            out_tiles = {}

            def emit_PV(st):
                c, m, sub, g, n_i = st["c"], st["m"], st["sub"], st["g"], st["n_i"]
                key = (c, m, sub)
                if st["first"]:
                    out_tiles[key] = outp.tile(
                        [128, 512], F32, tag="out", name=f"ov{c}_{m}{sub}"
                    )
                o_t = out_tiles[key]
                for j in range(2):
                    i = g + j
                    off = max(0, 128 * (i - 4 * c))
                    nc.tensor.matmul(
                        o_t[0:65, off:512],
                        lhsT=vaug[:, (sub * 16 + i) * 65 : (sub * 16 + i) * 65 + 65],
                        rhs=st["pr"][:, j * 512 + off : (j + 1) * 512],
                        start=(i == 0),
                        stop=(i == n_i - 1),
                        skip_group_check=True,
                    )

            def emit_norm(st):
                c, m, sub = st["c"], st["m"], st["sub"]
                hb = sub * 64
                o_t = out_tiles.pop((c, m, sub))
                rc = recp.tile([1, 512], F32, tag="rc", name=f"rc{c}_{m}{sub}")
                nc.vector.reciprocal(rc[:], o_t[64:65, :])
                rb = recp.tile([64, 512], F32, tag="rb", name=f"rb{c}_{m}{sub}")
                nc.gpsimd.partition_broadcast(rb[:], rc[:])
                nc.vector.tensor_tensor(
                    attn[hb : hb + 64, m * S + c * 512 : m * S + (c + 1) * 512],
                    o_t[0:64, :],
                    rb[:],
                    op=mybir.AluOpType.mult,
                )

            def emit_wo(st_idx):
                ot = otp.tile([128, 2048], F32, tag="ot", name=f"ot{st_idx}")
                for nk in range(4):
                    ps3 = ps3p.tile([128, 512], F32, tag="wo",
                                    name=f"wo{st_idx}_{nk}")
                    for kt in range(4):
                        nc.tensor.matmul(
                            ps3[:],
                            lhsT=attn[:, kt * S + st_idx * 128 : kt * S + st_idx * 128 + 128],
                            rhs=wo_sb[:, kt * 2048 + nk * 512 : kt * 2048 + (nk + 1) * 512],
                            start=(kt == 0),
                            stop=(kt == 3),
                            skip_group_check=True,
                        )
                    nc.vector.tensor_copy(ot[:, nk * 512 : (nk + 1) * 512], ps3[:])
                nc.sync.dma_start(
                    out_d[st_idx * 128 : (st_idx + 1) * 128, :], ot[:]
                )

            wo_queue = []
            prev = None
            for st in steps:
                emit_S(st)
                if prev is not None:
                    emit_PV(prev)
                    if prev["last"]:
                        emit_norm(prev)
                        if prev["blk_last"]:
                            c = prev["c"]
                            wo_queue.extend(4 * c + r for r in range(4))
                if st["first"] and st["bidx"] % 2 == 0 and wo_queue:
                    emit_wo(wo_queue.pop(0))
                prev = st
            emit_PV(prev)
            emit_norm(prev)
            wo_queue.extend(4 * prev["c"] + r for r in range(4))
            while wo_queue:
                emit_wo(wo_queue.pop(0))


def _get_compiled():
    global _COMPILED
    if _COMPILED is None:
        _COMPILED = _build()
    return _COMPILED


def _host_tables():
    invf = ROPE_BASE ** (-np.arange(0, DH, 2, dtype=np.float64) / DH)  # [32]
    t = np.arange(S, dtype=np.float64)
    theta = t[None, :] * invf[:, None]  # [32, S]
    c32 = np.cos(theta)
    s32 = np.sin(theta)
    C = np.empty((128, S), np.float32)
    Sg = np.empty((128, S), np.float32)
    for j in range(2):
        C[j * 64 : j * 64 + 32] = c32
        C[j * 64 + 32 : j * 64 + 64] = c32
        Sg[j * 64 : j * 64 + 32] = s32          # +sin for first half
        Sg[j * 64 + 32 : j * 64 + 64] = -s32    # -sin for second half
    tri = np.triu(np.ones((128, 128), np.float32))  # tri[a,b]=1 iff a<=b
    eye = np.tile(np.eye(64, dtype=np.float32), (2, 1))
    return C, Sg, tri, eye


# device head order within the 512-wide q shard: m-tile m holds local heads
# (m, m+4) so that the q sub-block partition base (64*sub) equals the kv head
# partition base.
_PERM_Q = np.array(
    [(m + 4 * sub) * DH + d for m in range(4) for sub in range(2) for d in range(DH)],
    dtype=np.int64,
)


def _rearrange_w(w):  # [2048, 768] -> [128, 12288] k-tile-major
    # device slice for (k, m) is w_dev[:, k*768 + m*128 : +128]
    return np.ascontiguousarray(
        w.reshape(16, 128, 768).transpose(1, 0, 2).reshape(128, 16 * 768)
    )


def _rearrange_wo(w):  # [512, 2048] -> [128, 8192]
    return np.ascontiguousarray(
        w.reshape(4, 128, 2048).transpose(1, 0, 2).reshape(128, 4 * 2048)
    )


def _rearrange_x(xb):  # [s, d] -> [128, 32768] chunk-major, k-tile, seq
    # device slice for chunk sc, k-tile k: xt[:, sc*8192 + k*512 : +512]
    return np.ascontiguousarray(
        xb.reshape(4, 512, 16, 128).transpose(3, 0, 2, 1).reshape(128, 4 * 8192)
    )


def _make_in_maps(ins):
    x = np.asarray(ins["x"], np.float32)
    wq = np.asarray(ins["wq"], np.float32)
    wk = np.asarray(ins["wk"], np.float32)
    wv = np.asarray(ins["wv"], np.float32)
    wo = np.asarray(ins["wo"], np.float32)

    C, Sg, tri, eye = _host_tables()
    C = C.astype(NPBF16)
    Sg = Sg.astype(NPBF16)
    tri = tri.astype(NPBF16)
    eye = eye.astype(NPBF16)
    xts = [_rearrange_x(x[bi]).astype(NPBF16) for bi in range(2)]

    in_maps = []
    for c in range(N_CORES):
        bi, g = c // 4, c % 4
        wq_s = wq[:, g * DQ : (g + 1) * DQ][:, _PERM_Q]
        wk_s = wk[:, g * DKV : (g + 1) * DKV]
        wv_s = wv[:, g * DKV : (g + 1) * DKV]
        wall = _rearrange_w(
            np.ascontiguousarray(np.concatenate([wq_s, wk_s, wv_s], axis=1))
        ).astype(NPBF16)
        wo_s = _rearrange_wo(
            np.ascontiguousarray(wo[g * DQ : (g + 1) * DQ, :][_PERM_Q])
        ).astype(NPBF16)
        in_maps.append(
            {
                "xt": xts[bi],
                "wall": wall,
                "wo": wo_s,
                "cos": C,
                "sin": Sg,
                "tri": tri,
                "eye": eye,
            }
        )
    return in_maps


def kernel(x, wq, wk, wv, wo):
    global LAST_RESULTS
    nc = _get_compiled()
    in_maps = _make_in_maps({"x": x, "wq": wq, "wk": wk, "wv": wv, "wo": wo})
    res = run_bass_kernel_spmd(nc, in_maps, list(range(N_CORES)), **RUN_KWARGS)
    LAST_RESULTS = res
    out = np.empty((2, S, D), np.float32)
    for bi in range(2):
        acc = res.results[4 * bi]["out"].astype(np.float32)
        for g in range(1, 4):
            acc = acc + res.results[4 * bi + g]["out"]
        out[bi] = acc
    return out
